# revision 24
# baseline (speedup 1.0000x reference)
"""Trainium2 Bass kernel for nn_AttMatch (2-graph attention + SAGEConv GNN).

Self-contained: takes the full unsharded inputs of the reference problem,
shards across 8 NeuronCores internally, runs one SPMD NEFF, and gathers the
full [8192, 8192] sigmoid adjacency output.

v2 restructure (from the AR-based baseline, 454666 ns):
  * softmax reduction via ONE ReduceScatter per (layer, graph) on
    [8, 129, 512] (segment c = core c's own 512 queries) instead of two
    AllReduces -- each core then normalizes only its own query slice,
    computes U = (lin_l-half of out) for its slice, and AllGathers U (fp8,
    64KB) for the SAGE aggregation.  Kills the 8x-redundant epilogue/ugen
    work and halves collective traffic.
  * per-(l,g) DRAM tensors for collective buffers -- no false WAR hazards
    (the baseline's shared ar_in tensor serialized attention ic2/ic3 DMA
    writes behind the previous half's AllReduce).
  * layer seams software-pipelined: tail_a(l,0) emitted as a filler inside
    attention(l,1); for layer 1 also xsage(1,0) and tail_b(1,0), so the
    serial CC chain (RS -> AG_U -> AG_h) overlaps attention; kv-projection
    split per graph so layer l+1 starts as soon as h(l,g) exists.
  * final sigmoid(F@F^T): graph-relative circulant 18 blocks/core
    (J00={0..4} g0->g0, J01={0..3} g0->g1, J10={1..4} g1->g0,
    J11={0..4} g1->g1), emitted in dependency phases (own-column blocks
    first) so blocks overlap the trailing AllGathers; sigmoid split
    scalar SIG (rt 0,1,3) / 4-op DVE u16-Schraudolph chain (rt 2):
    u16 bits = SIG16_B - SIG16_A*x bitcast as bf16 gives e^{-x} in one
    tensor_scalar (safe for |x|<266; final z measured in [-16, 70]).
  * warm-up AllReduce at kernel start absorbs cross-core launch stagger
    (60-130 us, the dominant run-to-run variance term).
  * NOT used (measured dead ends): gpsimd/Pool tensor ops (~7.4us per
    [128,512] op, 17x slower than DVE, cannot read PSUM); sync-queue
    (SP) dma_start (hangs, likely colliding with framework barriers).
"""

import numpy as np
import ml_dtypes

import concourse.bass as bass
import concourse.bacc as bacc
import concourse.tile as tile
import concourse.mybir as mybir
from concourse.bass_utils import run_bass_kernel_spmd

BF16 = ml_dtypes.bfloat16
E4M3 = ml_dtypes.float8_e4m3

N = 4096          # nodes per graph
D = 128           # feature dim
NCORES = 8
SH = N // NCORES  # 512 nodes per graph per core
ICW = 1024        # query-chunk width
NIC = N // ICW    # 4 query chunks
NT = 2 * SH // 128   # 8 local target tiles
NTP = NT // 2     # 4 target-tile pairs (DoubleRow)
NJ = N // 128     # 32 source-node tiles
NJP = NJ // 2     # 16 source-tile pairs
INV_SCALE = 1.0 / np.sqrt(128.0)

V_SCALE = 0.25         # v pre-scale (headroom in bf16/fp8 paths)
CS_SCALE = 1.0 / 64.0  # ones value for column sums
REP_SCALE = V_SCALE / CS_SCALE  # =16: makes prod = P/colsum exactly

LOG2E = 1.4426950408889634
ES_A = INV_SCALE * 8.0 * LOG2E
ES_B = 56.0 - 0.344 + 0.5
SIG_A = float(1 << 23) * LOG2E
SIG_B = 127.0 * float(1 << 23) - 360768.0
SIG_LO = -1.0e9
SIG_HI = 2.09e9
SIG16_A = SIG_A / 65536.0
SIG16_B = SIG_B / 65536.0 + 0.5

F32 = mybir.dt.float32
BF = mybir.dt.bfloat16
FP8 = mybir.dt.float8e4
U8 = mybir.dt.uint8
U16 = mybir.dt.uint16
I32 = mybir.dt.int32
DR = mybir.MatmulPerfMode.DoubleRow

ADD = mybir.AluOpType.add
SUB = mybir.AluOpType.subtract
MULT = mybir.AluOpType.mult
MAX = mybir.AluOpType.max
MIN = mybir.AluOpType.min
EXP = mybir.ActivationFunctionType.Exp
IDN = mybir.ActivationFunctionType.Identity
CPY = mybir.ActivationFunctionType.Copy
SIG = mybir.ActivationFunctionType.Sigmoid

WK, WQ, WV, WLS, WL1, WRS, WR1N = range(7)
IDENT = 14
BK, BQ, BV, BL = range(4)

_cache = {}


def _build_nc():
    nc = bacc.Bacc("TRN2", target_bir_lowering=False, debug=False,
                   num_devices=NCORES)

    # ---- external I/O ----
    x1t = nc.dram_tensor("x1t", [D, N], BF, kind="ExternalInput")
    x2t = nc.dram_tensor("x2t", [D, N], BF, kind="ExternalInput")
    xgt_in = [x1t, x2t]
    xown_in = nc.dram_tensor("xown", [2, D, SH], BF, kind="ExternalInput")
    mtc_in = [nc.dram_tensor("mtc1", [NJ, 128, SH], FP8, kind="ExternalInput"),
              nc.dram_tensor("mtc2", [NJ, 128, SH], FP8, kind="ExternalInput")]
    wm_in = nc.dram_tensor("wm", [128, 15 * 128], BF, kind="ExternalInput")
    bs_in = nc.dram_tensor("bs", [128, 8], F32, kind="ExternalInput")
    out_ext = nc.dram_tensor("out", [2, SH, 9 * 512], BF,
                             kind="ExternalOutput")

    # ---- internal DRAM for collectives (one tensor per use: no WAR) ----
    rg = [list(range(NCORES))]
    ar_in = [[nc.dram_tensor(f"ar_in_{l}_{g}", [NCORES, 129, 512], BF)
              for g in range(2)] for l in range(2)]
    rs_out = [[nc.dram_tensor(f"rs_out_{l}_{g}", [129, 512], BF)
               for g in range(2)] for l in range(2)]
    uag_in = [[nc.dram_tensor(f"uag_in_{l}_{g}", [128, 512], FP8)
               for g in range(2)] for l in range(2)]
    uag_out = [[nc.dram_tensor(f"uag_out_{l}_{g}", [NCORES, 128, 512], FP8,
                               addr_space="Shared")
                for g in range(2)] for l in range(2)]
    hag_in = [[nc.dram_tensor(f"hag_in_{l}_{g}", [D, SH], BF)
               for g in range(2)] for l in range(2)]
    hag_out = [[nc.dram_tensor(f"hag_out_{l}_{g}", [NCORES, D, SH], BF,
                               addr_space="Shared")
                for g in range(2)] for l in range(2)]
    wu_in = nc.dram_tensor("wu_in", [1, 64], BF)
    wu_out = nc.dram_tensor("wu_out", [1, 64], BF, addr_space="Shared")

    with tile.TileContext(nc) as tc:
        with (
            tc.tile_pool(name="const", bufs=1) as cpool,
            tc.tile_pool(name="mt", bufs=1) as mt_pool,
            tc.tile_pool(name="xt", bufs=1) as xt_pool,
            tc.tile_pool(name="xgd", bufs=1) as xgd_pool,
            tc.tile_pool(name="kq", bufs=2) as kq_pool,
            tc.tile_pool(name="es", bufs=4) as es_pool,
            tc.tile_pool(name="st", bufs=2) as st_pool,
            tc.tile_pool(name="tl", bufs=2) as tl_pool,
            tc.tile_pool(name="yu", bufs=1) as yu_pool,
            tc.tile_pool(name="sm", bufs=2) as sm_pool,
            tc.tile_pool(name="fz", bufs=8) as fz_pool,
            tc.tile_pool(name="psS", bufs=4, space="PSUM") as psS,
            tc.tile_pool(name="php", bufs=1, space="PSUM") as php_pool,
            tc.tile_pool(name="psC", bufs=2, space="PSUM") as psC,
        ):
            pid_pe = nc.tensor.partition_id()

            # ---- constants (host-packed partition-major) ----
            wm = cpool.tile([128, 15 * 128], BF, name="wm_sb")
            nc.scalar.dma_start(wm[:], wm_in[:])
            bs = cpool.tile([128, 8], F32, name="bs_sb")
            nc.scalar.dma_start(bs[:], bs_in[:])
            rep_row = cpool.tile([1, 128], BF, name="rep_row")
            nc.vector.memset(rep_row[:], REP_SCALE)
            ones8 = cpool.tile([128, 256], FP8, name="ones8")
            nc.vector.memset(ones8[:], CS_SCALE)
            # warm-up collective: absorbs cross-core startup skew
            wu = cpool.tile([1, 64], BF, name="wu")
            nc.vector.memset(wu[:], 0.0)
            nc.scalar.dma_start(wu_in[:], wu[:])
            nc.gpsimd.collective_compute(
                "AllReduce", ADD, replica_groups=rg,
                ins=[wu_in[:]], outs=[wu_out[:]])

            def W(l, i):
                base = 7 * l + i if i < 7 else IDENT
                return wm[:, 128 * base:128 * (base + 1)]

            def B(l, i):
                return bs[:, 4 * l + i:4 * l + i + 1]

            ident = wm[:, 128 * IDENT:128 * (IDENT + 1)]

            # ---- initial loads (xown first: unblocks kv-proj) ----
            xown = []
            for g in range(2):
                t = sm_pool.tile([D, SH], BF, name=f"xown{g}_0", tag=f"xo{g}")
                nc.scalar.dma_start(t[:], xown_in[g])
                xown.append(t)
            xgt = []
            for g in range(2):
                t = xt_pool.tile([D, N], BF, name=f"x{g}t_0", tag=f"xt{g}")
                nc.scalar.dma_start(t[:], xgt_in[g][:])
                xgt.append(t)
            mt = []
            for g in range(2):
                t = mt_pool.tile([128, NJ * SH], FP8, name=f"mt{g}")
                nc.scalar.dma_start(
                    t.rearrange("p (j n) -> p j n", j=NJ),
                    mtc_in[g].ap().rearrange("j p n -> p j n"))
                mt.append(t)

            state = {"xgt": xgt, "xown": xown}
            ls = [None, None]

            def proj_kv_alloc(l):
                kt = kq_pool.tile([D, 2 * SH], BF, name=f"kt_{l}", tag="kt")
                vnat = kq_pool.tile([128, NT * 128], FP8, name=f"vn_{l}",
                                    tag="vn")
                return kt, vnat

            def proj_kv_g(l, g, kt, vnat, xo):
                psk = psS.tile([128, 512], F32, tag="psS",
                               name=f"psk_{l}_{g}")
                nc.tensor.matmul(psk[:], W(l, WK), xo[:],
                                 start=True, stop=True)
                nc.vector.tensor_scalar(kt[:, g * SH:(g + 1) * SH],
                                        psk[:], B(l, BK), None, ADD)
                psv = psS.tile([128, 512], F32, tag="psS",
                               name=f"psv_{l}_{g}")
                nc.tensor.matmul(psv[:], W(l, WV), xo[:],
                                 start=True, stop=True)
                vt = st_pool.tile([128, SH], BF, name=f"vt_{l}_{g}",
                                  tag="vt")
                nc.vector.tensor_copy(vt[:], psv[:])
                pst = psC.tile([128, 512], BF, tag="psC",
                               name=f"pst_{l}_{g}")
                for j in range(4):
                    nc.tensor.transpose(pst[:, j * 128:(j + 1) * 128],
                                        vt[:, j * 128:(j + 1) * 128],
                                        ident)
                # v scaled by V_SCALE (fp8/bf16 range headroom)
                nc.vector.tensor_scalar(vnat[:, g * 512:(g + 1) * 512],
                                        pst[:], V_SCALE, None, MULT)

            def proj_q(l, g):
                q = kq_pool.tile([D, N], BF, name=f"qt_{l}_{g}",
                                 tag=f"qt{g}", bufs=1)
                for c in range(8):
                    psq = psS.tile([128, 512], F32, tag="psS",
                                   name=f"psq_{l}_{g}_{c}")
                    nc.tensor.matmul(psq[:], W(l, WQ),
                                     state["xgt"][g][:, c * 512:(c + 1) * 512],
                                     start=True, stop=True)
                    if c % 2 == 0:
                        nc.vector.tensor_scalar(
                            q[:, c * 512:(c + 1) * 512], psq[:],
                            B(l, BQ), None, ADD)
                    else:
                        nc.scalar.activation(
                            q[:, c * 512:(c + 1) * 512], psq[:], IDN,
                            bias=B(l, BQ))
                return q

            def attention(l, g, kt, vnat, qt, fillers=None):
                for ic in range(NIC):
                    php = php_pool.tile([128, 1024], F32, tag="php",
                                        name=f"php_{l}_{g}_{ic}")
                    pcs = [psC.tile([128, 512], F32, tag="psC",
                                    name=f"pcs{h}_{l}_{g}_{ic}")
                           for h in range(2)]
                    for tt2 in range(NTP):
                        es = [es_pool.tile([128, 1024], FP8, tag="es",
                                           name=f"es{h}_{l}_{g}_{ic}_{tt2}")
                              for h in range(2)]
                        for j in range(2):
                            tt = 2 * tt2 + j
                            for h in range(2):
                                ps_s = psS.tile(
                                    [128, 512], F32, tag="psS",
                                    name=f"pss_{l}_{g}_{ic}_{tt}_{h}")
                                nc.tensor.matmul(
                                    ps_s[:], kt[:, tt * 128:(tt + 1) * 128],
                                    qt[:, ic * ICW + h * 512:
                                          ic * ICW + (h + 1) * 512],
                                    start=True, stop=True)
                                dst = es[h][:, j * 512:(j + 1) * 512]
                                if h == 0:
                                    nc.vector.tensor_scalar(
                                        dst.bitcast(U8), ps_s[:],
                                        ES_A, ES_B, MULT, ADD)
                                else:
                                    nc.scalar.activation(dst, ps_s[:], EXP,
                                                         scale=INV_SCALE)
                        for h in range(2):
                            esh = es[h].rearrange("p (k n) -> p k n", k=2)
                            nc.tensor.matmul(
                                php[:, h * 512:(h + 1) * 512],
                                vnat[:, tt2 * 256:(tt2 + 1) * 256]
                                .rearrange("p (k m) -> p k m", k=2),
                                esh, start=(tt2 == 0), stop=(tt2 == NTP - 1),
                                perf_mode=DR)
                            nc.tensor.matmul(
                                pcs[h][:],
                                ones8.rearrange("p (k m) -> p k m", k=2),
                                esh, start=(tt2 == 0), stop=(tt2 == NTP - 1),
                                perf_mode=DR)
                    pc = st_pool.tile([128, ICW], BF, tag="pc")
                    cc = st_pool.tile([1, ICW], BF, tag="cc")
                    nc.vector.tensor_copy(pc[:, 0:512], php[:, 0:512])
                    nc.vector.tensor_copy(cc[:, 0:512], pcs[0][0:1, :])
                    nc.scalar.activation(pc[:, 512:1024], php[:, 512:1024],
                                         CPY)
                    nc.scalar.activation(cc[:, 512:1024], pcs[1][0:1, :],
                                         CPY)
                    for h in range(2):
                        seg = 2 * ic + h
                        nc.scalar.dma_start(ar_in[l][g][seg, 0:128, :],
                                          pc[:, h * 512:(h + 1) * 512])
                        nc.scalar.dma_start(ar_in[l][g][seg, 128:129, :],
                                          cc[:, h * 512:(h + 1) * 512])
                    if fillers is not None and ic in fillers:
                        fillers[ic]()
                nc.gpsimd.collective_compute(
                    "ReduceScatter", ADD, replica_groups=rg,
                    ins=[ar_in[l][g][:]], outs=[rs_out[l][g][:]])

            def xsage(l, g):
                xgt, xown = state["xgt"], state["xown"]
                yb = yu_pool.tile([128, N], FP8, name=f"yb_{l}_{g}", tag="yb")
                for jb in range(8):
                    psy = psS.tile([128, 512], F32, tag="psS",
                                   name=f"psy_{l}_{g}_{jb}")
                    for k in range(4):
                        jt = jb * 4 + k
                        nc.tensor.matmul(psy[:, k * 128:(k + 1) * 128],
                                         xgt[g][:, jt * 128:(jt + 1) * 128],
                                         W(l, WLS), start=True, stop=True)
                    if jb % 2 == 0:
                        nc.vector.tensor_copy(
                            yb[:, jb * 512:(jb + 1) * 512], psy[:])
                    else:
                        nc.scalar.activation(
                            yb[:, jb * 512:(jb + 1) * 512], psy[:], CPY)
                ps_a = psC.tile([128, 512], F32, tag="psC",
                                name=f"psa_{l}_{g}")
                for jp in range(NJP):
                    nc.tensor.matmul(
                        ps_a[:],
                        yb[:, jp * 256:(jp + 1) * 256]
                        .rearrange("p (k m) -> p k m", k=2),
                        mt[g][:, jp * 1024:(jp + 1) * 1024]
                        .rearrange("p (k n) -> p k n", k=2),
                        start=(jp == 0), stop=False, perf_mode=DR,
                        skip_group_check=True)
                nc.tensor.matmul(ps_a[:], W(l, WRS), xown[g][:],
                                 start=False, stop=True,
                                 skip_group_check=True)
                t = sm_pool.tile([128, SH], F32, name=f"ls_{l}_{g}",
                                 tag=f"ls{g}", bufs=1)
                nc.vector.tensor_scalar(t[:], ps_a[:], B(l, BL), None, ADD)
                ls[g] = t

            def tail_a(l, g):
                """Own-slice epilogue: normalize own 512 queries, build
                U-own, launch the U AllGather.  Cheap enough to interleave
                mid-attention."""
                pf = tl_pool.tile([128, 512], BF, name=f"pf_{l}_{g}",
                                  tag="pf")
                csr = tl_pool.tile([1, 512], BF, name=f"csr_{l}_{g}",
                                   tag="csr")
                nc.gpsimd.dma_start(pf[:], rs_out[l][g][0:128, :])
                nc.gpsimd.dma_start(csr[:], rs_out[l][g][128:129, :])
                ps_rep = psS.tile([128, 512], F32, tag="psS",
                                  name=f"psrep_{l}_{g}")
                nc.tensor.matmul(ps_rep[:], rep_row[:], csr[:],
                                 start=True, stop=True)
                rr = st_pool.tile([128, 512], F32, name=f"rr_{l}_{g}",
                                  tag="rr")
                nc.vector.reciprocal_approx_fast(rr[:], ps_rep[:])
                prod = st_pool.tile([128, 512], BF, name=f"prod_{l}_{g}",
                                    tag="prod")
                nc.vector.tensor_tensor(prod[:], pf[:], rr[:], MULT)
                outt = tl_pool.tile([128, 512], BF, name=f"outt_{l}_{g}",
                                    tag="outt")
                nc.scalar.activation(outt[:], prod[:], IDN, bias=B(l, BV))
                psu = psS.tile([128, 512], F32, tag="psS",
                               name=f"psu_{l}_{g}")
                for k in range(4):
                    nc.tensor.matmul(psu[:, k * 128:(k + 1) * 128],
                                     outt[:, k * 128:(k + 1) * 128],
                                     W(l, WL1), start=True, stop=True)
                ubo = st_pool.tile([128, 512], FP8, name=f"ubo_{l}_{g}",
                                   tag="ubo")
                nc.scalar.activation(ubo[:], psu[:], CPY)
                nc.scalar.dma_start(uag_in[l][g][:], ubo[:])
                nc.gpsimd.collective_compute(
                    "AllGather", mybir.AluOpType.bypass, replica_groups=rg,
                    ins=[uag_in[l][g][:]], outs=[uag_out[l][g][:]])
                return outt

            def tail_b(l, g, outt):
                """Aggregate U over the graph, finish h, AllGather h."""
                ubf = yu_pool.tile([128, N], FP8, name=f"ubf_{l}_{g}",
                                   tag="ub")
                for c in range(NCORES):
                    nc.gpsimd.dma_start(ubf[:, c * 512:(c + 1) * 512],
                                        uag_out[l][g][c])
                ps_a2 = psC.tile([128, 512], F32, tag="psC",
                                 name=f"psa2_{l}_{g}")
                for jp in range(NJP):
                    nc.tensor.matmul(
                        ps_a2[:],
                        ubf[:, jp * 256:(jp + 1) * 256]
                        .rearrange("p (k m) -> p k m", k=2),
                        mt[g][:, jp * 1024:(jp + 1) * 1024]
                        .rearrange("p (k n) -> p k n", k=2),
                        start=(jp == 0), stop=False, perf_mode=DR,
                        skip_group_check=True)
                nc.tensor.matmul(ps_a2[:], W(l, WR1N), outt[:],
                                 start=False, stop=True,
                                 skip_group_check=True)
                h = sm_pool.tile([D, SH], BF, name=f"hown_{l}_{g}",
                                 tag=f"xo{g}")
                if l == 0:
                    t2 = st_pool.tile([128, 512], F32, name=f"t2_{l}_{g}",
                                      tag="t2")
                    nc.vector.tensor_tensor(t2[:], ls[g][:], ps_a2[:], SUB)
                    nc.vector.tensor_scalar(h[:], t2[:], 0.0, None, MAX)
                else:
                    nc.vector.tensor_tensor(h[:], ls[g][:], ps_a2[:], SUB)
                nc.scalar.dma_start(hag_in[l][g][:], h[:])
                nc.gpsimd.collective_compute(
                    "AllGather", mybir.AluOpType.bypass, replica_groups=rg,
                    ins=[hag_in[l][g][:]], outs=[hag_out[l][g][:]])
                if l == 0:
                    t = xt_pool.tile([D, N], BF, name=f"x{g}t_1",
                                     tag=f"xt{g}")
                    for c in range(NCORES):
                        nc.gpsimd.dma_start(t[:, c * 512:(c + 1) * 512],
                                            hag_out[l][g][c])
                    state["xgt"][g] = t
                return h

            # ========== final-block helpers ==========
            def sig_chain_dve(ps_z, z):
                # u16 Schraudolph: bf16 bit pattern of e^{-x} in one op
                u = fz_pool.tile([128, 512], BF, tag="sg")
                nc.vector.tensor_scalar(u[:].bitcast(U16), ps_z[:],
                                        -SIG16_A, SIG16_B, MULT, ADD)
                v = fz_pool.tile([128, 512], F32, tag="sg")
                nc.vector.tensor_scalar(v[:], u[:], 1.0, None, ADD)
                r = fz_pool.tile([128, 512], F32, tag="sg")
                nc.vector.reciprocal_approx_fast(r[:], v[:])
                nc.vector.tensor_copy(z[:], r[:])

            def fin_block(grow, slot, lhs, rhs, blki):
                z2 = fz_pool.tile([128, 4 * 512], BF, tag="z2", bufs=3)
                for rt in range(4):
                    ps_z = psS.tile([128, 512], F32, tag="psS",
                                    name=f"psz_{grow}_{slot}_{rt}")
                    nc.tensor.matmul(
                        ps_z[:], lhs[:, rt * 128:(rt + 1) * 128],
                        rhs, start=True, stop=True)
                    zs = z2[:, rt * 512:(rt + 1) * 512]
                    if rt == 2:
                        sig_chain_dve(ps_z, zs)
                    else:
                        nc.scalar.activation(zs, ps_z[:], SIG)
                nc.scalar.dma_start(
                    out_ext[grow].rearrange("(r p) n -> p r n", r=4)
                    [:, :, slot * 512:(slot + 1) * 512],
                    z2.rearrange("p (r n) -> p r n", r=4))

            # ================= layers =================
            # layer 0 (no fillers in attn(0,1): RS(0,0) completion is
            # gated by cross-core startup skew, so consume it as late as
            # possible)
            kt0, vn0 = proj_kv_alloc(0)
            proj_kv_g(0, 0, kt0, vn0, xown[0])
            proj_kv_g(0, 1, kt0, vn0, xown[1])
            q00 = proj_q(0, 0)
            attention(0, 0, kt0, vn0, q00)
            q01 = proj_q(0, 1)
            ot0 = {}
            attention(0, 1, kt0, vn0, q01,
                      fillers={1: lambda: ot0.__setitem__(0, tail_a(0, 0))})
            xsage(0, 0)
            h00 = tail_b(0, 0, ot0[0])
            kt1, vn1 = proj_kv_alloc(1)
            proj_kv_g(1, 0, kt1, vn1, h00)
            xsage(0, 1)
            ot01 = tail_a(0, 1)
            q10 = proj_q(1, 0)
            h01 = tail_b(0, 1, ot01)
            proj_kv_g(1, 1, kt1, vn1, h01)
            state["xown"] = [h00, h01]
            # layer 1: tail work of (1,0) interleaved into attn(1,1)
            attention(1, 0, kt1, vn1, q10)
            q11 = proj_q(1, 1)
            fst = {}

            def fill_ic2():
                fst["ot"] = tail_a(1, 0)

            attention(1, 1, kt1, vn1, q11,
                      fillers={0: lambda: xsage(1, 0),
                               1: fill_ic2})
            # tail_b(1,0) AFTER attention: its AG_h(1,0) trigger now follows
            # RS(1,1) on the CC queue, so the h11 chain starts ~15us earlier
            fst["h"] = tail_b(1, 0, fst["ot"])
            xg0d = xgd_pool.tile([128, 2 * N], BF, name="xg0d")
            for r in range(2):
                for c in range(NCORES):
                    nc.gpsimd.dma_start(
                        xg0d[:, r * N + c * 512:r * N + (c + 1) * 512],
                        hag_out[1][0][c])
            fst["xg0d"] = xg0d
            h10, xg0d = fst["h"], fst["xg0d"]
            # phase alpha: own g0 column (needs only h10)
            fin_block(0, 0, h10, h10[:], 0)
            xsage(1, 1)
            # gamma row-0 blocks (lhs h10, rhs xg0d) fill the RS(1,1) and
            # AG_U(1,1) waits
            bi = 1
            for j in (1, 2):
                fin_block(0, j, h10,
                          xg0d[:, bass.ds((pid_pe + j) * 512, 512)], bi)
                bi += 1
            ot11 = tail_a(1, 1)
            for j in (3, 4):
                fin_block(0, j, h10,
                          xg0d[:, bass.ds((pid_pe + j) * 512, 512)], bi)
                bi += 1
            h11 = tail_b(1, 1, ot11)
            # xg1d load issued immediately (gpsimd DGE does not head-of-line
            # block); data arrives while beta/gamma-row1 compute
            xg1d = xgd_pool.tile([128, 2 * N], BF, name="xg1d")
            for r in range(2):
                for c in range(NCORES):
                    nc.gpsimd.dma_start(
                        xg1d[:, r * N + c * 512:r * N + (c + 1) * 512],
                        hag_out[1][1][c])
            # phase beta: own g1 columns (need h11)
            fin_block(0, 5, h10, h11[:], bi)
            bi += 1
            fin_block(1, 0, h11, h11[:], bi)
            bi += 1
            # gamma row-1: lhs h11, rhs xg0d (J10 = {1..7}; only 4 blocks
            # remain dependent on the last AllGather)
            for j in range(1, 5):
                fin_block(1, 4 + j, h11,
                          xg0d[:, bass.ds((pid_pe + j) * 512, 512)], bi)
                bi += 1
            for j in range(5, 8):
                fin_block(0, j + 1, h11,
                          xg0d[:, bass.ds((pid_pe + j) * 512, 512)], bi)
                bi += 1
            # phase delta: rhs from graph-1 AllGather (J11 j=1..4)
            for j in range(1, 5):
                fin_block(1, j, h11,
                          xg1d[:, bass.ds((pid_pe + j) * 512, 512)], bi)
                bi += 1

    nc.compile()
    return nc


def _host_prep(inputs):
    """Build per-core input maps from the full problem inputs."""
    x1 = np.asarray(inputs["x1"], np.float32)
    x2 = np.asarray(inputs["x2"], np.float32)
    x1t = np.ascontiguousarray(x1.T).astype(BF16)
    x2t = np.ascontiguousarray(x2.T).astype(BF16)

    def norm_adj_t(ei):
        ei = np.asarray(ei)
        A = np.zeros((N, N), np.float32)
        np.add.at(A, (ei[1], ei[0]), 1.0)
        deg = A.sum(1)
        A /= np.maximum(deg, 1.0)[:, None]
        return np.ascontiguousarray(A.T)  # MT[j, n]

    mt = [norm_adj_t(inputs["ei1"]), norm_adj_t(inputs["ei2"])]

    wm = np.zeros((15, 128, 128), np.float32)
    bs = np.zeros((8, 128, 1), np.float32)
    for l, s in enumerate(("1", "2")):
        wm[7 * l + WK] = inputs["Wk" + s]
        wm[7 * l + WQ] = inputs["Wq" + s]
        wm[7 * l + WV] = inputs["Wv" + s]
        wm[7 * l + WLS] = inputs["Wl" + s][:128] + inputs["Wl" + s][128:]
        wm[7 * l + WL1] = inputs["Wl" + s][128:]
        wm[7 * l + WRS] = inputs["Wr" + s][:128] + inputs["Wr" + s][128:]
        wm[7 * l + WR1N] = inputs["Wr" + s][128:]
        bs[4 * l + BK, :, 0] = inputs["bk" + s]
        bs[4 * l + BQ, :, 0] = inputs["bq" + s]
        bs[4 * l + BV, :, 0] = inputs["bv" + s]
        bs[4 * l + BL, :, 0] = inputs["bl" + s]
    wm[IDENT] = np.eye(128)
    wm_p = np.ascontiguousarray(
        wm.transpose(1, 0, 2).reshape(128, 15 * 128)).astype(BF16)
    bs_p = np.ascontiguousarray(bs[:, :, 0].T).astype(np.float32)

    in_maps = []
    for c in range(NCORES):
        sl = slice(c * SH, (c + 1) * SH)
        in_maps.append({
            "x1t": x1t,
            "x2t": x2t,
            "xown": np.stack([x1t[:, sl], x2t[:, sl]]),
            "mtc1": np.ascontiguousarray(
                mt[0][:, sl]).astype(E4M3).reshape(NJ, 128, SH),
            "mtc2": np.ascontiguousarray(
                mt[1][:, sl]).astype(E4M3).reshape(NJ, 128, SH),
            "wm": wm_p,
            "bs": bs_p,
        })
    return in_maps


def _slot_rc(c):
    """(row_unit, col_unit) for each of the 9 slots, per output plane."""
    rc0 = [(c, c)] + [(c, (c + j) % 8) for j in range(1, 5)] + \
        [(c, 8 + c)] + [(8 + c, (c + j) % 8) for j in range(5, 8)]
    rc1 = [(8 + c, 8 + c)] + [(8 + c, 8 + (c + j) % 8) for j in range(1, 5)] + \
        [(8 + c, (c + j) % 8) for j in range(1, 5)]
    return [rc0, rc1]


def _assemble(results):
    """Place each core's 18 circulant blocks, mirror the rest."""
    full = np.empty((2 * N, 2 * N), np.float32)
    filled = np.zeros((16, 16), bool)
    for c in range(NCORES):
        o = np.asarray(results[c]["out"]).astype(np.float32)
        rcs = _slot_rc(c)
        for gi in range(2):
            for slot in range(9):
                ru, cu = rcs[gi][slot]
                full[ru * 512:(ru + 1) * 512, cu * 512:(cu + 1) * 512] = \
                    o[gi][:, slot * 512:(slot + 1) * 512]
                filled[ru, cu] = True
    for a in range(16):
        for b in range(16):
            if not filled[a, b]:
                full[a * 512:(a + 1) * 512, b * 512:(b + 1) * 512] = \
                    full[b * 512:(b + 1) * 512, a * 512:(a + 1) * 512].T
    return full


def get_nc():
    if "nc" not in _cache:
        _cache["nc"] = _build_nc()
    return _cache["nc"]


def kernel(**inputs):
    nc = get_nc()
    in_maps = _host_prep(inputs)
    res = run_bass_kernel_spmd(nc, in_maps, core_ids=list(range(NCORES)))
    return _assemble(res.results)


# revision 25
# speedup vs baseline: 1.0215x; 1.0215x over previous
"""Trainium2 Bass kernel for nn_AttMatch (2-graph attention + SAGEConv GNN).

Self-contained: takes the full unsharded inputs of the reference problem,
shards across 8 NeuronCores internally, runs one SPMD NEFF, and gathers the
full [8192, 8192] sigmoid adjacency output.

v2 restructure (from the AR-based baseline, 454666 ns):
  * softmax reduction via ONE ReduceScatter per (layer, graph) on
    [8, 129, 512] (segment c = core c's own 512 queries) instead of two
    AllReduces -- each core then normalizes only its own query slice,
    computes U = (lin_l-half of out) for its slice, and AllGathers U (fp8,
    64KB) for the SAGE aggregation.  Kills the 8x-redundant epilogue/ugen
    work and halves collective traffic.
  * per-(l,g) DRAM tensors for collective buffers -- no false WAR hazards
    (the baseline's shared ar_in tensor serialized attention ic2/ic3 DMA
    writes behind the previous half's AllReduce).
  * layer seams software-pipelined: tail_a(l,0) emitted as a filler inside
    attention(l,1); for layer 1 also xsage(1,0) and tail_b(1,0), so the
    serial CC chain (RS -> AG_U -> AG_h) overlaps attention; kv-projection
    split per graph so layer l+1 starts as soon as h(l,g) exists.
  * final sigmoid(F@F^T): graph-relative circulant 18 blocks/core
    (J00={0..4} g0->g0, J01={0..3} g0->g1, J10={1..4} g1->g0,
    J11={0..4} g1->g1), emitted in dependency phases (own-column blocks
    first) so blocks overlap the trailing AllGathers; sigmoid split
    scalar SIG (rt 0,1,3) / 4-op DVE u16-Schraudolph chain (rt 2):
    u16 bits = SIG16_B - SIG16_A*x bitcast as bf16 gives e^{-x} in one
    tensor_scalar (safe for |x|<266; final z measured in [-16, 70]).
  * warm-up AllReduce at kernel start absorbs cross-core launch stagger
    (60-130 us, the dominant run-to-run variance term).
  * NOT used (measured dead ends): gpsimd/Pool tensor ops (~7.4us per
    [128,512] op, 17x slower than DVE, cannot read PSUM); sync-queue
    (SP) dma_start (hangs, likely colliding with framework barriers).
"""

import numpy as np
import ml_dtypes

import concourse.bass as bass
import concourse.bacc as bacc
import concourse.tile as tile
import concourse.mybir as mybir
from concourse.bass_utils import run_bass_kernel_spmd

BF16 = ml_dtypes.bfloat16
E4M3 = ml_dtypes.float8_e4m3

N = 4096          # nodes per graph
D = 128           # feature dim
NCORES = 8
SH = N // NCORES  # 512 nodes per graph per core
ICW = 1024        # query-chunk width
NIC = N // ICW    # 4 query chunks
NT = 2 * SH // 128   # 8 local target tiles
NTP = NT // 2     # 4 target-tile pairs (DoubleRow)
NJ = N // 128     # 32 source-node tiles
NJP = NJ // 2     # 16 source-tile pairs
INV_SCALE = 1.0 / np.sqrt(128.0)

V_SCALE = 0.25         # v pre-scale (headroom in bf16/fp8 paths)
CS_SCALE = 1.0 / 64.0  # ones value for column sums
REP_SCALE = V_SCALE / CS_SCALE  # =16: makes prod = P/colsum exactly

LOG2E = 1.4426950408889634
ES_A = INV_SCALE * 8.0 * LOG2E
ES_B = 56.0 - 0.344 + 0.5
SIG_A = float(1 << 23) * LOG2E
SIG_B = 127.0 * float(1 << 23) - 360768.0
SIG_LO = -1.0e9
SIG_HI = 2.09e9
SIG16_A = SIG_A / 65536.0
SIG16_B = SIG_B / 65536.0 + 0.5

F32 = mybir.dt.float32
BF = mybir.dt.bfloat16
FP8 = mybir.dt.float8e4
U8 = mybir.dt.uint8
U16 = mybir.dt.uint16
I32 = mybir.dt.int32
DR = mybir.MatmulPerfMode.DoubleRow

ADD = mybir.AluOpType.add
SUB = mybir.AluOpType.subtract
MULT = mybir.AluOpType.mult
MAX = mybir.AluOpType.max
MIN = mybir.AluOpType.min
EXP = mybir.ActivationFunctionType.Exp
IDN = mybir.ActivationFunctionType.Identity
CPY = mybir.ActivationFunctionType.Copy
SIG = mybir.ActivationFunctionType.Sigmoid

WK, WQ, WV, WLS, WL1, WRS, WR1N = range(7)
IDENT = 14
BK, BQ, BV, BL = range(4)

_cache = {}


def _build_nc():
    nc = bacc.Bacc("TRN2", target_bir_lowering=False, debug=False,
                   num_devices=NCORES)

    # ---- external I/O ----
    x1t = nc.dram_tensor("x1t", [D, N], BF, kind="ExternalInput")
    x2t = nc.dram_tensor("x2t", [D, N], BF, kind="ExternalInput")
    xgt_in = [x1t, x2t]
    xown_in = nc.dram_tensor("xown", [2, D, SH], BF, kind="ExternalInput")
    mtc_in = [nc.dram_tensor("mtc1", [NJ, 128, SH], FP8, kind="ExternalInput"),
              nc.dram_tensor("mtc2", [NJ, 128, SH], FP8, kind="ExternalInput")]
    wm_in = nc.dram_tensor("wm", [128, 15 * 128], BF, kind="ExternalInput")
    bs_in = nc.dram_tensor("bs", [128, 8], F32, kind="ExternalInput")
    out_ext = nc.dram_tensor("out", [2, SH, 9 * 512], BF,
                             kind="ExternalOutput")

    # ---- internal DRAM for collectives (one tensor per use: no WAR) ----
    rg = [list(range(NCORES))]
    ar_in = [[nc.dram_tensor(f"ar_in_{l}_{g}", [NCORES, 129, 512], BF)
              for g in range(2)] for l in range(2)]
    rs_out = [[nc.dram_tensor(f"rs_out_{l}_{g}", [129, 512], BF)
               for g in range(2)] for l in range(2)]
    uag_in = [[nc.dram_tensor(f"uag_in_{l}_{g}", [128, 512], FP8)
               for g in range(2)] for l in range(2)]
    uag_out = [[nc.dram_tensor(f"uag_out_{l}_{g}", [NCORES, 128, 512], FP8,
                               addr_space="Shared")
                for g in range(2)] for l in range(2)]
    hag_in = [[nc.dram_tensor(f"hag_in_{l}_{g}", [D, SH], BF)
               for g in range(2)] for l in range(2)]
    hag_out = [[nc.dram_tensor(f"hag_out_{l}_{g}", [NCORES, D, SH], BF,
                               addr_space="Shared")
                for g in range(2)] for l in range(2)]
    wu_in = nc.dram_tensor("wu_in", [1, 64], BF)
    wu_out = nc.dram_tensor("wu_out", [1, 64], BF, addr_space="Shared")

    with tile.TileContext(nc) as tc:
        with (
            tc.tile_pool(name="const", bufs=1) as cpool,
            tc.tile_pool(name="mt", bufs=1) as mt_pool,
            tc.tile_pool(name="xt", bufs=1) as xt_pool,
            tc.tile_pool(name="xgd", bufs=1) as xgd_pool,
            tc.tile_pool(name="kq", bufs=2) as kq_pool,
            tc.tile_pool(name="es", bufs=4) as es_pool,
            tc.tile_pool(name="st", bufs=2) as st_pool,
            tc.tile_pool(name="tl", bufs=2) as tl_pool,
            tc.tile_pool(name="yu", bufs=1) as yu_pool,
            tc.tile_pool(name="sm", bufs=2) as sm_pool,
            tc.tile_pool(name="fz", bufs=8) as fz_pool,
            tc.tile_pool(name="psS", bufs=4, space="PSUM") as psS,
            tc.tile_pool(name="php", bufs=1, space="PSUM") as php_pool,
            tc.tile_pool(name="psC", bufs=2, space="PSUM") as psC,
        ):
            pid_pe = nc.tensor.partition_id()

            # ---- constants (host-packed partition-major) ----
            wm = cpool.tile([128, 15 * 128], BF, name="wm_sb")
            nc.scalar.dma_start(wm[:], wm_in[:])
            bs = cpool.tile([128, 8], F32, name="bs_sb")
            nc.scalar.dma_start(bs[:], bs_in[:])
            rep_row = cpool.tile([1, 128], BF, name="rep_row")
            nc.vector.memset(rep_row[:], REP_SCALE)
            ones8 = cpool.tile([128, 256], FP8, name="ones8")
            nc.vector.memset(ones8[:], CS_SCALE)
            # warm-up collective: absorbs cross-core startup skew
            wu = cpool.tile([1, 64], BF, name="wu")
            nc.vector.memset(wu[:], 0.0)
            nc.scalar.dma_start(wu_in[:], wu[:])
            nc.gpsimd.collective_compute(
                "AllReduce", ADD, replica_groups=rg,
                ins=[wu_in[:]], outs=[wu_out[:]])

            def W(l, i):
                base = 7 * l + i if i < 7 else IDENT
                return wm[:, 128 * base:128 * (base + 1)]

            def B(l, i):
                return bs[:, 4 * l + i:4 * l + i + 1]

            ident = wm[:, 128 * IDENT:128 * (IDENT + 1)]

            # ---- initial loads (xown first: unblocks kv-proj) ----
            xown = []
            for g in range(2):
                t = sm_pool.tile([D, SH], BF, name=f"xown{g}_0", tag=f"xo{g}")
                nc.scalar.dma_start(t[:], xown_in[g])
                xown.append(t)
            xgt = []
            for g in range(2):
                t = xt_pool.tile([D, N], BF, name=f"x{g}t_0", tag=f"xt{g}")
                nc.scalar.dma_start(t[:], xgt_in[g][:])
                xgt.append(t)
            mt = []
            for g in range(2):
                t = mt_pool.tile([128, NJ * SH], FP8, name=f"mt{g}")
                nc.scalar.dma_start(
                    t.rearrange("p (j n) -> p j n", j=NJ),
                    mtc_in[g].ap().rearrange("j p n -> p j n"))
                mt.append(t)

            state = {"xgt": xgt, "xown": xown}
            ls = [None, None]

            def proj_kv_alloc(l):
                kt = kq_pool.tile([D, 2 * SH], BF, name=f"kt_{l}", tag="kt")
                vnat = kq_pool.tile([128, NT * 128], FP8, name=f"vn_{l}",
                                    tag="vn")
                return kt, vnat

            def proj_kv_g(l, g, kt, vnat, xo):
                psk = psS.tile([128, 512], F32, tag="psS",
                               name=f"psk_{l}_{g}")
                nc.tensor.matmul(psk[:], W(l, WK), xo[:],
                                 start=True, stop=True)
                nc.vector.tensor_scalar(kt[:, g * SH:(g + 1) * SH],
                                        psk[:], B(l, BK), None, ADD)
                psv = psS.tile([128, 512], F32, tag="psS",
                               name=f"psv_{l}_{g}")
                nc.tensor.matmul(psv[:], W(l, WV), xo[:],
                                 start=True, stop=True)
                vt = st_pool.tile([128, SH], BF, name=f"vt_{l}_{g}",
                                  tag="vt")
                nc.vector.tensor_copy(vt[:], psv[:])
                pst = psC.tile([128, 512], BF, tag="psC",
                               name=f"pst_{l}_{g}")
                for j in range(4):
                    nc.tensor.transpose(pst[:, j * 128:(j + 1) * 128],
                                        vt[:, j * 128:(j + 1) * 128],
                                        ident)
                # v scaled by V_SCALE (fp8/bf16 range headroom)
                nc.vector.tensor_scalar(vnat[:, g * 512:(g + 1) * 512],
                                        pst[:], V_SCALE, None, MULT)

            def proj_q(l, g):
                q = kq_pool.tile([D, N], BF, name=f"qt_{l}_{g}",
                                 tag=f"qt{g}", bufs=1)
                for c in range(8):
                    psq = psS.tile([128, 512], F32, tag="psS",
                                   name=f"psq_{l}_{g}_{c}")
                    nc.tensor.matmul(psq[:], W(l, WQ),
                                     state["xgt"][g][:, c * 512:(c + 1) * 512],
                                     start=True, stop=True)
                    if c % 2 == 0:
                        nc.vector.tensor_scalar(
                            q[:, c * 512:(c + 1) * 512], psq[:],
                            B(l, BQ), None, ADD)
                    else:
                        nc.scalar.activation(
                            q[:, c * 512:(c + 1) * 512], psq[:], IDN,
                            bias=B(l, BQ))
                return q

            def attention(l, g, kt, vnat, qt, fillers=None):
                for ic in range(NIC):
                    php = php_pool.tile([128, 1024], F32, tag="php",
                                        name=f"php_{l}_{g}_{ic}")
                    pcs = [psC.tile([128, 512], F32, tag="psC",
                                    name=f"pcs{h}_{l}_{g}_{ic}")
                           for h in range(2)]
                    for tt2 in range(NTP):
                        es = [es_pool.tile([128, 1024], FP8, tag="es",
                                           name=f"es{h}_{l}_{g}_{ic}_{tt2}")
                              for h in range(2)]
                        for j in range(2):
                            tt = 2 * tt2 + j
                            for h in range(2):
                                ps_s = psS.tile(
                                    [128, 512], F32, tag="psS",
                                    name=f"pss_{l}_{g}_{ic}_{tt}_{h}")
                                nc.tensor.matmul(
                                    ps_s[:], kt[:, tt * 128:(tt + 1) * 128],
                                    qt[:, ic * ICW + h * 512:
                                          ic * ICW + (h + 1) * 512],
                                    start=True, stop=True)
                                dst = es[h][:, j * 512:(j + 1) * 512]
                                if h == 0:
                                    nc.vector.tensor_scalar(
                                        dst.bitcast(U8), ps_s[:],
                                        ES_A, ES_B, MULT, ADD)
                                else:
                                    nc.scalar.activation(dst, ps_s[:], EXP,
                                                         scale=INV_SCALE)
                        for h in range(2):
                            esh = es[h].rearrange("p (k n) -> p k n", k=2)
                            nc.tensor.matmul(
                                php[:, h * 512:(h + 1) * 512],
                                vnat[:, tt2 * 256:(tt2 + 1) * 256]
                                .rearrange("p (k m) -> p k m", k=2),
                                esh, start=(tt2 == 0), stop=(tt2 == NTP - 1),
                                perf_mode=DR)
                            nc.tensor.matmul(
                                pcs[h][:],
                                ones8.rearrange("p (k m) -> p k m", k=2),
                                esh, start=(tt2 == 0), stop=(tt2 == NTP - 1),
                                perf_mode=DR)
                    pc = st_pool.tile([128, ICW], BF, tag="pc")
                    cc = st_pool.tile([1, ICW], BF, tag="cc")
                    nc.vector.tensor_copy(pc[:, 0:512], php[:, 0:512])
                    nc.vector.tensor_copy(cc[:, 0:512], pcs[0][0:1, :])
                    nc.scalar.activation(pc[:, 512:1024], php[:, 512:1024],
                                         CPY)
                    nc.scalar.activation(cc[:, 512:1024], pcs[1][0:1, :],
                                         CPY)
                    for h in range(2):
                        seg = 2 * ic + h
                        nc.scalar.dma_start(ar_in[l][g][seg, 0:128, :],
                                          pc[:, h * 512:(h + 1) * 512])
                        nc.scalar.dma_start(ar_in[l][g][seg, 128:129, :],
                                          cc[:, h * 512:(h + 1) * 512])
                    if fillers is not None and ic in fillers:
                        fillers[ic]()
                nc.gpsimd.collective_compute(
                    "ReduceScatter", ADD, replica_groups=rg,
                    ins=[ar_in[l][g][:]], outs=[rs_out[l][g][:]])

            def xsage(l, g):
                xgt, xown = state["xgt"], state["xown"]
                yb = yu_pool.tile([128, N], FP8, name=f"yb_{l}_{g}", tag="yb")
                for jb in range(8):
                    psy = psS.tile([128, 512], F32, tag="psS",
                                   name=f"psy_{l}_{g}_{jb}")
                    for k in range(4):
                        jt = jb * 4 + k
                        nc.tensor.matmul(psy[:, k * 128:(k + 1) * 128],
                                         xgt[g][:, jt * 128:(jt + 1) * 128],
                                         W(l, WLS), start=True, stop=True)
                    if jb % 2 == 0:
                        nc.vector.tensor_copy(
                            yb[:, jb * 512:(jb + 1) * 512], psy[:])
                    else:
                        nc.scalar.activation(
                            yb[:, jb * 512:(jb + 1) * 512], psy[:], CPY)
                ps_a = psC.tile([128, 512], F32, tag="psC",
                                name=f"psa_{l}_{g}")
                for jp in range(NJP):
                    nc.tensor.matmul(
                        ps_a[:],
                        yb[:, jp * 256:(jp + 1) * 256]
                        .rearrange("p (k m) -> p k m", k=2),
                        mt[g][:, jp * 1024:(jp + 1) * 1024]
                        .rearrange("p (k n) -> p k n", k=2),
                        start=(jp == 0), stop=False, perf_mode=DR,
                        skip_group_check=True)
                nc.tensor.matmul(ps_a[:], W(l, WRS), xown[g][:],
                                 start=False, stop=True,
                                 skip_group_check=True)
                t = sm_pool.tile([128, SH], F32, name=f"ls_{l}_{g}",
                                 tag=f"ls{g}", bufs=1)
                nc.vector.tensor_scalar(t[:], ps_a[:], B(l, BL), None, ADD)
                ls[g] = t

            def tail_a(l, g):
                """Own-slice epilogue: normalize own 512 queries, build
                U-own, launch the U AllGather.  Cheap enough to interleave
                mid-attention."""
                pf = tl_pool.tile([128, 512], BF, name=f"pf_{l}_{g}",
                                  tag="pf")
                csr = tl_pool.tile([1, 512], BF, name=f"csr_{l}_{g}",
                                   tag="csr")
                nc.gpsimd.dma_start(pf[:], rs_out[l][g][0:128, :])
                nc.gpsimd.dma_start(csr[:], rs_out[l][g][128:129, :])
                ps_rep = psS.tile([128, 512], F32, tag="psS",
                                  name=f"psrep_{l}_{g}")
                nc.tensor.matmul(ps_rep[:], rep_row[:], csr[:],
                                 start=True, stop=True)
                rr = st_pool.tile([128, 512], F32, name=f"rr_{l}_{g}",
                                  tag="rr")
                nc.vector.reciprocal_approx_fast(rr[:], ps_rep[:])
                prod = st_pool.tile([128, 512], BF, name=f"prod_{l}_{g}",
                                    tag="prod")
                nc.vector.tensor_tensor(prod[:], pf[:], rr[:], MULT)
                outt = tl_pool.tile([128, 512], BF, name=f"outt_{l}_{g}",
                                    tag="outt")
                nc.scalar.activation(outt[:], prod[:], IDN, bias=B(l, BV))
                psu = psS.tile([128, 512], F32, tag="psS",
                               name=f"psu_{l}_{g}")
                for k in range(4):
                    nc.tensor.matmul(psu[:, k * 128:(k + 1) * 128],
                                     outt[:, k * 128:(k + 1) * 128],
                                     W(l, WL1), start=True, stop=True)
                ubo = st_pool.tile([128, 512], FP8, name=f"ubo_{l}_{g}",
                                   tag="ubo")
                nc.scalar.activation(ubo[:], psu[:], CPY)
                nc.scalar.dma_start(uag_in[l][g][:], ubo[:])
                nc.gpsimd.collective_compute(
                    "AllGather", mybir.AluOpType.bypass, replica_groups=rg,
                    ins=[uag_in[l][g][:]], outs=[uag_out[l][g][:]])
                return outt

            def tail_b(l, g, outt):
                """Aggregate U over the graph, finish h, AllGather h."""
                ubf = yu_pool.tile([128, N], FP8, name=f"ubf_{l}_{g}",
                                   tag="ub")
                for c in range(NCORES):
                    nc.gpsimd.dma_start(ubf[:, c * 512:(c + 1) * 512],
                                        uag_out[l][g][c])
                ps_a2 = psC.tile([128, 512], F32, tag="psC",
                                 name=f"psa2_{l}_{g}")
                for jp in range(NJP):
                    nc.tensor.matmul(
                        ps_a2[:],
                        ubf[:, jp * 256:(jp + 1) * 256]
                        .rearrange("p (k m) -> p k m", k=2),
                        mt[g][:, jp * 1024:(jp + 1) * 1024]
                        .rearrange("p (k n) -> p k n", k=2),
                        start=(jp == 0), stop=False, perf_mode=DR,
                        skip_group_check=True)
                nc.tensor.matmul(ps_a2[:], W(l, WR1N), outt[:],
                                 start=False, stop=True,
                                 skip_group_check=True)
                h = sm_pool.tile([D, SH], BF, name=f"hown_{l}_{g}",
                                 tag=f"xo{g}")
                if l == 0:
                    t2 = st_pool.tile([128, 512], F32, name=f"t2_{l}_{g}",
                                      tag="t2")
                    nc.vector.tensor_tensor(t2[:], ls[g][:], ps_a2[:], SUB)
                    nc.vector.tensor_scalar(h[:], t2[:], 0.0, None, MAX)
                else:
                    nc.vector.tensor_tensor(h[:], ls[g][:], ps_a2[:], SUB)
                nc.scalar.dma_start(hag_in[l][g][:], h[:])
                nc.gpsimd.collective_compute(
                    "AllGather", mybir.AluOpType.bypass, replica_groups=rg,
                    ins=[hag_in[l][g][:]], outs=[hag_out[l][g][:]])
                if l == 0:
                    t = xt_pool.tile([D, N], BF, name=f"x{g}t_1",
                                     tag=f"xt{g}")
                    for c in range(NCORES):
                        nc.gpsimd.dma_start(t[:, c * 512:(c + 1) * 512],
                                            hag_out[l][g][c])
                    state["xgt"][g] = t
                return h

            # ========== final-block helpers ==========
            def sig_chain_dve(ps_z, z):
                # u16 Schraudolph: bf16 bit pattern of e^{-x} in one op
                u = fz_pool.tile([128, 512], BF, tag="sg")
                nc.vector.tensor_scalar(u[:].bitcast(U16), ps_z[:],
                                        -SIG16_A, SIG16_B, MULT, ADD)
                v = fz_pool.tile([128, 512], F32, tag="sg")
                nc.vector.tensor_scalar(v[:], u[:], 1.0, None, ADD)
                r = fz_pool.tile([128, 512], F32, tag="sg")
                nc.vector.reciprocal_approx_fast(r[:], v[:])
                nc.vector.tensor_copy(z[:], r[:])

            def fin_block(grow, slot, lhs, rhs, blki):
                z2 = fz_pool.tile([128, 4 * 512], BF, tag="z2", bufs=3)
                for rt in range(4):
                    ps_z = psS.tile([128, 512], F32, tag="psS",
                                    name=f"psz_{grow}_{slot}_{rt}")
                    nc.tensor.matmul(
                        ps_z[:], lhs[:, rt * 128:(rt + 1) * 128],
                        rhs, start=True, stop=True)
                    zs = z2[:, rt * 512:(rt + 1) * 512]
                    if rt == 2:
                        sig_chain_dve(ps_z, zs)
                    else:
                        nc.scalar.activation(zs, ps_z[:], SIG)
                nc.scalar.dma_start(
                    out_ext[grow].rearrange("(r p) n -> p r n", r=4)
                    [:, :, slot * 512:(slot + 1) * 512],
                    z2.rearrange("p (r n) -> p r n", r=4))

            # ================= layers =================
            # layer 0 (no fillers in attn(0,1): RS(0,0) completion is
            # gated by cross-core startup skew, so consume it as late as
            # possible)
            kt0, vn0 = proj_kv_alloc(0)
            proj_kv_g(0, 0, kt0, vn0, xown[0])
            proj_kv_g(0, 1, kt0, vn0, xown[1])
            q00 = proj_q(0, 0)
            attention(0, 0, kt0, vn0, q00)
            q01 = proj_q(0, 1)
            ot0 = {}
            attention(0, 1, kt0, vn0, q01,
                      fillers={1: lambda: ot0.__setitem__(0, tail_a(0, 0))})
            xsage(0, 0)
            h00 = tail_b(0, 0, ot0[0])
            kt1, vn1 = proj_kv_alloc(1)
            proj_kv_g(1, 0, kt1, vn1, h00)
            xsage(0, 1)
            ot01 = tail_a(0, 1)
            q10 = proj_q(1, 0)
            h01 = tail_b(0, 1, ot01)
            proj_kv_g(1, 1, kt1, vn1, h01)
            state["xown"] = [h00, h01]
            # layer 1: tail work of (1,0) interleaved into attn(1,1)
            attention(1, 0, kt1, vn1, q10)
            q11 = proj_q(1, 1)
            fst = {}

            def fill_ic2():
                fst["ot"] = tail_a(1, 0)

            def fill_ic3():
                fst["h"] = tail_b(1, 0, fst["ot"])
                xg0d = xgd_pool.tile([128, 2 * N], BF, name="xg0d")
                for r in range(2):
                    for c in range(NCORES):
                        nc.gpsimd.dma_start(
                            xg0d[:, r * N + c * 512:r * N + (c + 1) * 512],
                            hag_out[1][0][c])
                fst["xg0d"] = xg0d

            attention(1, 1, kt1, vn1, q11,
                      fillers={0: lambda: xsage(1, 0),
                               1: fill_ic2, 2: fill_ic3})
            h10, xg0d = fst["h"], fst["xg0d"]
            # phase alpha: own g0 column (needs only h10)
            fin_block(0, 0, h10, h10[:], 0)
            xsage(1, 1)
            # gamma row-0 blocks (lhs h10, rhs xg0d) fill the RS(1,1) and
            # AG_U(1,1) waits
            bi = 1
            for j in (1, 2):
                fin_block(0, j, h10,
                          xg0d[:, bass.ds((pid_pe + j) * 512, 512)], bi)
                bi += 1
            ot11 = tail_a(1, 1)
            for j in (3, 4):
                fin_block(0, j, h10,
                          xg0d[:, bass.ds((pid_pe + j) * 512, 512)], bi)
                bi += 1
            h11 = tail_b(1, 1, ot11)
            # xg1d load issued immediately (gpsimd DGE does not head-of-line
            # block); data arrives while beta/gamma-row1 compute
            xg1d = xgd_pool.tile([128, 2 * N], BF, name="xg1d")
            for r in range(2):
                for c in range(NCORES):
                    nc.gpsimd.dma_start(
                        xg1d[:, r * N + c * 512:r * N + (c + 1) * 512],
                        hag_out[1][1][c])
            # phase beta: own g1 columns (need h11)
            fin_block(0, 5, h10, h11[:], bi)
            bi += 1
            fin_block(1, 0, h11, h11[:], bi)
            bi += 1
            # gamma row-1: lhs h11, rhs xg0d (J10 = {1..7}; only 4 blocks
            # remain dependent on the last AllGather)
            for j in range(1, 5):
                fin_block(1, 4 + j, h11,
                          xg0d[:, bass.ds((pid_pe + j) * 512, 512)], bi)
                bi += 1
            for j in range(5, 8):
                fin_block(0, j + 1, h11,
                          xg0d[:, bass.ds((pid_pe + j) * 512, 512)], bi)
                bi += 1
            # phase delta: rhs from graph-1 AllGather (J11 j=1..4)
            for j in range(1, 5):
                fin_block(1, j, h11,
                          xg1d[:, bass.ds((pid_pe + j) * 512, 512)], bi)
                bi += 1

    nc.compile()
    return nc


def _host_prep(inputs):
    """Build per-core input maps from the full problem inputs."""
    x1 = np.asarray(inputs["x1"], np.float32)
    x2 = np.asarray(inputs["x2"], np.float32)
    x1t = np.ascontiguousarray(x1.T).astype(BF16)
    x2t = np.ascontiguousarray(x2.T).astype(BF16)

    def norm_adj_t(ei):
        ei = np.asarray(ei)
        A = np.zeros((N, N), np.float32)
        np.add.at(A, (ei[1], ei[0]), 1.0)
        deg = A.sum(1)
        A /= np.maximum(deg, 1.0)[:, None]
        return np.ascontiguousarray(A.T)  # MT[j, n]

    mt = [norm_adj_t(inputs["ei1"]), norm_adj_t(inputs["ei2"])]

    wm = np.zeros((15, 128, 128), np.float32)
    bs = np.zeros((8, 128, 1), np.float32)
    for l, s in enumerate(("1", "2")):
        wm[7 * l + WK] = inputs["Wk" + s]
        wm[7 * l + WQ] = inputs["Wq" + s]
        wm[7 * l + WV] = inputs["Wv" + s]
        wm[7 * l + WLS] = inputs["Wl" + s][:128] + inputs["Wl" + s][128:]
        wm[7 * l + WL1] = inputs["Wl" + s][128:]
        wm[7 * l + WRS] = inputs["Wr" + s][:128] + inputs["Wr" + s][128:]
        wm[7 * l + WR1N] = inputs["Wr" + s][128:]
        bs[4 * l + BK, :, 0] = inputs["bk" + s]
        bs[4 * l + BQ, :, 0] = inputs["bq" + s]
        bs[4 * l + BV, :, 0] = inputs["bv" + s]
        bs[4 * l + BL, :, 0] = inputs["bl" + s]
    wm[IDENT] = np.eye(128)
    wm_p = np.ascontiguousarray(
        wm.transpose(1, 0, 2).reshape(128, 15 * 128)).astype(BF16)
    bs_p = np.ascontiguousarray(bs[:, :, 0].T).astype(np.float32)

    in_maps = []
    for c in range(NCORES):
        sl = slice(c * SH, (c + 1) * SH)
        in_maps.append({
            "x1t": x1t,
            "x2t": x2t,
            "xown": np.stack([x1t[:, sl], x2t[:, sl]]),
            "mtc1": np.ascontiguousarray(
                mt[0][:, sl]).astype(E4M3).reshape(NJ, 128, SH),
            "mtc2": np.ascontiguousarray(
                mt[1][:, sl]).astype(E4M3).reshape(NJ, 128, SH),
            "wm": wm_p,
            "bs": bs_p,
        })
    return in_maps


def _slot_rc(c):
    """(row_unit, col_unit) for each of the 9 slots, per output plane."""
    rc0 = [(c, c)] + [(c, (c + j) % 8) for j in range(1, 5)] + \
        [(c, 8 + c)] + [(8 + c, (c + j) % 8) for j in range(5, 8)]
    rc1 = [(8 + c, 8 + c)] + [(8 + c, 8 + (c + j) % 8) for j in range(1, 5)] + \
        [(8 + c, (c + j) % 8) for j in range(1, 5)]
    return [rc0, rc1]


def _assemble(results):
    """Place each core's 18 circulant blocks, mirror the rest."""
    full = np.empty((2 * N, 2 * N), np.float32)
    filled = np.zeros((16, 16), bool)
    for c in range(NCORES):
        o = np.asarray(results[c]["out"]).astype(np.float32)
        rcs = _slot_rc(c)
        for gi in range(2):
            for slot in range(9):
                ru, cu = rcs[gi][slot]
                full[ru * 512:(ru + 1) * 512, cu * 512:(cu + 1) * 512] = \
                    o[gi][:, slot * 512:(slot + 1) * 512]
                filled[ru, cu] = True
    for a in range(16):
        for b in range(16):
            if not filled[a, b]:
                full[a * 512:(a + 1) * 512, b * 512:(b + 1) * 512] = \
                    full[b * 512:(b + 1) * 512, a * 512:(a + 1) * 512].T
    return full


def get_nc():
    if "nc" not in _cache:
        _cache["nc"] = _build_nc()
    return _cache["nc"]


def kernel(**inputs):
    nc = get_nc()
    in_maps = _host_prep(inputs)
    res = run_bass_kernel_spmd(nc, in_maps, core_ids=list(range(NCORES)))
    return _assemble(res.results)
